# revision 33
# baseline (speedup 1.0000x reference)
"""Causal self-attention Trainium2 kernel.

B=4, T=2048, C=1024, H=16 heads (D=64). 8 NeuronCores.

Sharding (hybrid data/tensor parallel, Megatron-style):
  core i -> (batch b = i//2, head-group g = i%2 of 8 heads).
  c_attn column-parallel (each core owns its group's q/k/v columns),
  c_proj row-parallel (each core owns its group's rows); the 2 partial
  outputs per batch are summed on the host (host-side all-reduce),
  b_proj added once at the end.

Per-core device kernel (T=2048 tokens, 8 heads, D=64):
  A1: qT/kT produced in [D, T] layout (weights stationary, xT streaming).
  A2: V produced interleaved [tok, d|1, h] with a ones column per head
      (softmax denominators fused into PV as an extra output row).
  B:  per (head pair, q block): S^T[k,q] tiles = kT.T @ qT (K=64 matmul,
      two heads packed in row groups 0-63 / 64-127), exp on ScalarE
      straight out of PSUM (no max subtraction: logits are ~N(0,1)),
      causal masking via one gpsimd affine_select per diagonal half
      (zero-fills both the fully-masked strip and the triangle), then
      O^T_aug[d|denom, q] += [V|1].T @ P^T accumulated over k tiles.
      Normalization: 1/denom via the fast custom-DVE reciprocal
      (reciprocal_approx_fast), gpsimd partition_broadcast, DVE mul.
  C:  out[t, c] = Onorm^T.T @ wo accumulated over 4 channel tiles.

Emission order pipelines the phases so TensorE never idles long enough
for the HAM clock gate to re-throttle: A1(ct0) first, then qb-major
B blocks with the A2 chunk for that qb ahead of them, A1(ct) woven in
right before its first B block, and C chunks emitted as soon as the
last head pair of a q block completes.
"""

import sys

import numpy as np

sys.path.insert(0, "/opt/trn_rl_repo")

from contextlib import ExitStack

import concourse.bacc as bacc
import concourse.tile as tile
from concourse import mybir
from concourse.bass_utils import run_bass_kernel_spmd

F32 = mybir.dt.float32
BF16 = mybir.dt.bfloat16

B, T, C, H = 4, 2048, 1024, 16
D = C // H            # 64 head dim
G = 2                 # head groups (cores per batch)
NH = H // G           # 8 heads per core
CH = NH * D           # 512 channels per core
N_CORES = B * G       # 8

KT = C // 128         # 8 contraction tiles for qkv proj
TB = T // 512         # 4 token blocks of 512
CT = NH // 2          # 4 channel tiles (head pairs)
TT = T // 128         # 16 token tiles of 128
CB = C // 512         # 2 output channel blocks
QB = T // 512         # 4 q blocks
SCALE = 1.0 / float(np.sqrt(D))

_last_results = None  # BassKernelResults of the most recent kernel() call


def _build_program(include_bias: bool) -> bacc.Bacc:
    nc = bacc.Bacc("TRN2")

    xT = nc.dram_tensor("xT", [C, T], BF16, kind="ExternalInput").ap()
    wq = nc.dram_tensor("wq", [C, CH], BF16, kind="ExternalInput").ap()
    wk = nc.dram_tensor("wk", [C, CH], BF16, kind="ExternalInput").ap()
    wv = nc.dram_tensor("wv", [C, CH], BF16, kind="ExternalInput").ap()
    wo = nc.dram_tensor("wo", [CH, C], BF16, kind="ExternalInput").ap()
    if include_bias:
        bq = nc.dram_tensor("bq", [CH], BF16, kind="ExternalInput").ap()
        bk = nc.dram_tensor("bk", [CH], BF16, kind="ExternalInput").ap()
        bv = nc.dram_tensor("bv", [CH], BF16, kind="ExternalInput").ap()
    out = nc.dram_tensor("out", [T, C], F32, kind="ExternalOutput").ap()

    with tile.TileContext(nc) as tc, ExitStack() as ctx:
        persist = ctx.enter_context(tc.tile_pool(name="persist", bufs=1))
        # [D, T] layouts, one tile per head pair: rows 0-63 head 2*ct,
        # rows 64-127 head 2*ct+1.
        qT = [persist.tile([128, T], BF16, name=f"qT{i}", tag=f"qT{i}") for i in range(CT)]
        kTs = [persist.tile([128, T], BF16, name=f"kT{i}", tag=f"kT{i}") for i in range(CT)]
        # V interleaved h-major: vint[tt][p, h, d] = V[t=128*tt+p, head h,
        # dim d], with vint[tt][p, h, D] = 1.0 (denominator column). Head
        # slices [:, h, :] are contiguous so PV's LDWEIGHTS streams fast.
        vint = [persist.tile([128, NH, D + 1], BF16, name=f"v{i}", tag=f"v{i}") for i in range(TT)]
        # Normalized attention output, [ch, T] layout per head pair.
        onorm = [persist.tile([128, T], BF16, name=f"on{i}", tag=f"on{i}") for i in range(CT)]
        ones_row = persist.tile([1, 512], BF16, name="ones", tag="ones")
        nc.vector.memset(ones_row, 1.0)
        if include_bias:
            bias_sb = persist.tile([1, 3, CH], BF16, name="bias", tag="bias")
            nc.sync.dma_start(
                out=bias_sb[:, 0, :], in_=bq.rearrange("(a c) -> a c", a=1)
            )
            nc.sync.dma_start(
                out=bias_sb[:, 1, :], in_=bk.rearrange("(a c) -> a c", a=1)
            )
            nc.sync.dma_start(
                out=bias_sb[:, 2, :], in_=bv.rearrange("(a c) -> a c", a=1)
            )

        # Weight + xT residency (everything stays in SBUF for the whole
        # kernel so projection matmuls can interleave with attention).
        xT_sb = [
            persist.tile([128, T], BF16, name=f"xT{k}", tag=f"xT{k}")
            for k in range(KT)
        ]
        wq_sb = [persist.tile([128, CH], BF16, name=f"wq{k}", tag=f"wq{k}") for k in range(KT)]
        wk_sb = [persist.tile([128, CH], BF16, name=f"wk{k}", tag=f"wk{k}") for k in range(KT)]
        wv_sb = [persist.tile([128, CH], BF16, name=f"wv{k}", tag=f"wv{k}") for k in range(KT)]
        wo_sb = [
            persist.tile([128, C], BF16, name=f"wo{i}", tag=f"wo{i}")
            for i in range(CT)
        ]
        # DMA order: the A1(ct0) k-chunks first so its matmuls start early.
        # Spread the 8.5MB input load across four DMA trigger queues so
        # the prologue is bandwidth- rather than queue-bound: xT (the
        # critical 4MB, gating the first qT/kT blocks) alternates between
        # the sync and vector queues, q/k weights go on scalar, and the
        # later-needed wv/wo ride the slow SWDGE queue.
        for k in range(KT):
            src = xT[k * 128 : (k + 1) * 128, :]
            if k % 2 == 0:
                nc.sync.dma_start(out=xT_sb[k], in_=src)
            else:
                nc.scalar.dma_start(out=xT_sb[k], in_=src)
            nc.sync.dma_start(out=wq_sb[k], in_=wq[k * 128 : (k + 1) * 128, :])
            nc.scalar.dma_start(out=wk_sb[k], in_=wk[k * 128 : (k + 1) * 128, :])
        for k in range(KT):
            nc.gpsimd.dma_start(out=wv_sb[k], in_=wv[k * 128 : (k + 1) * 128, :])
        for ct in range(CT):
            nc.gpsimd.dma_start(
                out=wo_sb[ct], in_=wo[ct * 128 : (ct + 1) * 128, :]
            )
        for tt in range(TT):
            nc.gpsimd.memset(vint[tt][:, :, D], 1.0)
        # Preload the exp activation table during the DMA prologue so the
        # first real exp doesn't pay the ~2.7us table load.
        warm_act = persist.tile([1, 8], BF16, name="wact", tag="wact")
        nc.scalar.activation(
            warm_act, ones_row[:, 0:8], mybir.ActivationFunctionType.Exp
        )

        # Shared PSUM pools. Budget (8 banks): spool 2x2 + opool 2x1 +
        # aux 2x1 = 8.
        auxps = ctx.enter_context(tc.tile_pool(name="auxps", bufs=2, space="PSUM"))
        spool = ctx.enter_context(tc.tile_pool(name="spool", bufs=2, space="PSUM"))
        opool = ctx.enter_context(tc.tile_pool(name="opool", bufs=2, space="PSUM"))
        ptpool = ctx.enter_context(tc.tile_pool(name="ptpool", bufs=16))
        rpool = ctx.enter_context(tc.tile_pool(name="rpool", bufs=5))
        bcpool = ctx.enter_context(tc.tile_pool(name="bcpool", bufs=3))
        stpool = ctx.enter_context(tc.tile_pool(name="stpool", bufs=3))
        ostage = ctx.enter_context(tc.tile_pool(name="ostage", bufs=4))
        costage = ctx.enter_context(tc.tile_pool(name="costage", bufs=3))

        def emit_a1_unit(ct, bi, tb):
            # One qT/kT 512-token block for head pair ct; weights reloaded
            # per block (LDWEIGHTS hides in the background weight buffer).
            wsb, dest = ((wq_sb, qT), (wk_sb, kTs))[bi]
            ps = auxps.tile([128, 512], F32, name="a1", tag="aux")
            for k in range(KT):
                nc.tensor.matmul(
                    ps,
                    lhsT=wsb[k][:, ct * 128 : (ct + 1) * 128],
                    rhs=xT_sb[k][:, tb * 512 : (tb + 1) * 512],
                    start=(k == 0),
                    stop=(k == KT - 1 and not include_bias),
                )
            if include_bias:
                nc.tensor.matmul(
                    ps,
                    lhsT=bias_sb[:, bi, ct * 128 : (ct + 1) * 128],
                    rhs=ones_row,
                    start=False,
                    stop=True,
                )
            nc.vector.tensor_copy(dest[ct][:, tb * 512 : (tb + 1) * 512], ps)

        def emit_a2(tt):
            # V chunk for token tile tt, interleaved layout + ones column.
            ps = auxps.tile([128, 512], F32, name="a2", tag="aux")
            for k in range(KT):
                nc.tensor.matmul(
                    ps,
                    lhsT=xT_sb[k][:, tt * 128 : (tt + 1) * 128],
                    rhs=wv_sb[k],
                    start=(k == 0),
                    stop=(k == KT - 1 and not include_bias),
                )
            if include_bias:
                nc.tensor.matmul(
                    ps,
                    lhsT=ones_row[:, 0:128],
                    rhs=bias_sb[:, 2, :],
                    start=False,
                    stop=True,
                )
            nc.vector.tensor_copy(
                vint[tt][:, :, 0:D],
                ps.rearrange("p (h d) -> p h d", h=NH),
            )

        def emit_s(ct, qb, kp, pts):
            # S^T matmuls for both heads of the pair; per k-tile the two
            # K=64 matmuls land in different PE row groups.
            ps_pair = []
            for hh in range(2):
                ps_pair.append(spool.tile([128, 1024], F32, name="s", tag="s"))
            for half in range(2):
                kt = 2 * kp + half
                j = kt - 4 * qb
                # Diagonal tiles: q columns < 128j are fully masked — skip
                # them in the matmul (the affine_select already treats that
                # region as a fill zone, so downstream logic is unchanged).
                off = 128 * j if j > 0 else 0
                for hh in range(2):
                    rb = 64 * hh
                    nc.tensor.matmul(
                        ps_pair[hh][:, half * 512 + off : (half + 1) * 512],
                        lhsT=kTs[ct][rb : rb + 64, kt * 128 : (kt + 1) * 128],
                        rhs=qT[ct][rb : rb + 64, qb * 512 + off : (qb + 1) * 512],
                        start=True,
                        stop=True,
                        tile_position=(rb, 0),
                    )
            j0 = 2 * kp - 4 * qb  # diag offset of first half (<0: below)
            for hh in range(2):
                ps_s = ps_pair[hh]
                pt = ptpool.tile([128, 1024], BF16, name="pt", tag="pt")
                if j0 <= 0:
                    # Fully below the diagonal (j0 < 0), or diag pair A
                    # (j0 == 0: only 128 masked cols — cheaper to exp them
                    # and zero-fill than to split the ACT).
                    nc.scalar.activation(
                        pt, ps_s, mybir.ActivationFunctionType.Exp,
                        scale=SCALE,
                    )
                else:
                    # diag pair B (j0 == 2): halves j=2, j=3; skip the
                    # large fully-masked strips in the ACT.
                    nc.scalar.activation(
                        pt[:, 256:512], ps_s[:, 256:512],
                        mybir.ActivationFunctionType.Exp, scale=SCALE,
                    )
                    nc.scalar.activation(
                        pt[:, 896:1024], ps_s[:, 896:1024],
                        mybir.ActivationFunctionType.Exp, scale=SCALE,
                    )
                if j0 >= 0:
                    # Triangle-only select: PV's N-trim skips the fully
                    # masked strip (cols < 128j of the half), so only the
                    # 128-wide diagonal chunk needs masking — keep
                    # pt[ch, c] iff c - ch >= 0 within the chunk.
                    for half in range(2):
                        j = j0 + half
                        o = half * 512 + 128 * j
                        nc.gpsimd.affine_select(
                            out=pt[:, o : o + 128],
                            in_=pt[:, o : o + 128],
                            compare_op=mybir.AluOpType.is_ge,
                            fill=0.0,
                            base=0,
                            channel_multiplier=-1,
                            pattern=[[1, 128]],
                        )
                pts[(kp, hh)] = pt

        def emit_pv(ct, qb, kp, nkt, oaug, pts):
            for hh in range(2):
                h = 2 * ct + hh
                pt = pts.pop((kp, hh))
                for half in range(2):
                    kt = 2 * kp + half
                    j = kt - 4 * qb
                    # Diagonal tiles contribute nothing to q cols < 128j;
                    # skip those output columns (they were initialized by
                    # the kt==0 matmul, which is never trimmed).
                    off = 128 * j if j > 0 else 0
                    nc.tensor.matmul(
                        oaug[hh][:, off:512],
                        lhsT=vint[kt][:, h, :],
                        rhs=pt[:, half * 512 + off : (half + 1) * 512],
                        start=(kt == 0),
                        stop=(kt == nkt - 1),
                    )

        def flush_norm(pending):
            # Normalization of an earlier block, deferred so its gpsimd
            # partition_broadcast queues BEHIND the next block's
            # affine_selects (no head-of-line blocking of PV).
            if pending is None:
                return
            ocps, ct, qb = pending
            qs = slice(qb * 512, (qb + 1) * 512)
            for hh in range(2):
                ocp, dstg = ocps[hh]
                rc = rpool.tile([1, 512], F32, name="r", tag="r")
                nc.vector.reciprocal_approx_fast(rc, dstg)
                bc = bcpool.tile([64, 512], F32, name="bc", tag="bc")
                nc.gpsimd.partition_broadcast(bc, rc, channels=64)
                if hh == 0:
                    nc.vector.tensor_mul(
                        onorm[ct][0:64, qs], ocp[0:D, :], bc
                    )
                else:
                    stg = stpool.tile([64, 512], BF16, name="st", tag="st")
                    nc.vector.tensor_mul(stg, ocp[0:D, :], bc)
                    nc.sync.dma_start(out=onorm[ct][64:128, qs], in_=stg)

        def emit_b_block(ct, qb, pending):
            nkt = 4 * qb + 4  # causal: only k tiles with k <= q
            nkp = nkt // 2
            # Strict mode separation: all 64-row S matmuls first (T0/T8
            # row-tile concurrency), then all 128-row PV matmuls —
            # interleaving the two tile modes forces a TensorE drain per
            # switch, and the Tile scheduler re-interleaves by readiness
            # anyway (verified fastest of the orderings tried).
            pts = {}
            for kp in range(nkp):
                emit_s(ct, qb, kp, pts)
            flush_norm(pending)
            if ct == 0:
                # V chunk for this q block, needed by PV(0, qb) onward.
                for tt in range(4 * qb, 4 * qb + 4):
                    emit_a2(tt)
            oaug = [
                opool.tile([D + 1, 512], F32, name=f"oaug{hh}", tag="oaug")
                for hh in range(2)
            ]
            for kp in range(nkp):
                emit_pv(ct, qb, kp, nkt, oaug, pts)
            # Evacuate O_aug to SBUF immediately: frees the PSUM bank for
            # the next block; normalization runs off the SBUF copy later.
            # Evacuate to SBUF immediately (frees the PSUM bank for the
            # next block): O rows partition-aligned, the denominator row
            # via a separate plain copy (partition 64 -> 0; the custom-DVE
            # reciprocal cannot handle a cross-partition source itself).
            ocps = []
            for hh in range(2):
                ocp = ostage.tile([D, 512], F32, name="oc", tag="oc")
                nc.vector.tensor_copy(ocp, oaug[hh][0:D, :])
                dstg = rpool.tile([1, 512], F32, name="d", tag="d")
                nc.vector.tensor_copy(dstg, oaug[hh][D : D + 1, :])
                ocps.append((ocp, dstg))
            return (ocps, ct, qb)

        def emit_c(tt):
            # out[tt block] = sum_ct onorm[ct]^T @ wo[ct]
            pcs = [
                auxps.tile([128, 512], F32, name="c", tag="aux")
                for _ in range(CB)
            ]
            for ct in range(CT):
                for cb in range(CB):
                    nc.tensor.matmul(
                        pcs[cb],
                        lhsT=onorm[ct][:, tt * 128 : (tt + 1) * 128],
                        rhs=wo_sb[ct][:, cb * 512 : (cb + 1) * 512],
                        start=(ct == 0),
                        stop=(ct == CT - 1),
                    )
            for cb in range(CB):
                ot = costage.tile([128, 512], F32, name="o", tag="o")
                # Split the PSUM evacuations between the two engines that
                # can read PSUM so neither becomes the pole.
                if cb == 0:
                    nc.vector.tensor_copy(ot, pcs[cb])
                else:
                    nc.scalar.copy(ot, pcs[cb])
                nc.sync.dma_start(
                    out=out[
                        tt * 128 : (tt + 1) * 128,
                        cb * 512 : (cb + 1) * 512,
                    ],
                    in_=ot,
                )

        # ---------------- pipelined emission ----------------
        # Zig-zag block order, tuned so the cumulative exp supply to
        # ScalarE tracks cumulative TensorE work: A1(0) units pairwise
        # feed B(0,qb) blocks (first exp ~DMA-bound), A1(ct) quads woven
        # as filler, late B blocks ordered large-qb-first to keep the exp
        # stream dense, C chunks as soon as their last block lands.
        SEQ = [
            ("u", 0, 0, 0), ("u", 0, 1, 0), ("b", 0, 0),
            ("u", 0, 0, 1), ("u", 0, 1, 1), ("b", 0, 1),
            ("uq", 1, 0),
            ("u", 0, 0, 2), ("u", 0, 1, 2), ("b", 0, 2),
            ("uq", 1, 1),
            ("u", 0, 0, 3), ("u", 0, 1, 3), ("b", 0, 3),
            ("b", 1, 3), ("b", 1, 2),
            ("uq", 2, 0),
            ("b", 1, 1),
            ("uq", 2, 1),
            ("b", 1, 0),
            ("b", 2, 3),
            ("uq", 3, 0),
            ("b", 2, 2),
            ("uq", 3, 1),
            ("b", 3, 3), ("c", 3),
            ("b", 2, 1),
            ("b", 3, 2), ("c", 2),
            ("b", 2, 0),
            ("b", 3, 1), ("c", 1),
            ("b", 3, 0), ("c", 0),
        ]
        pending = None
        for item in SEQ:
            kind = item[0]
            if kind == "u":
                _, ct, bi, tb = item
                emit_a1_unit(ct, bi, tb)
            elif kind == "uq":
                _, ct, q = item
                for u in range(4 * q, 4 * q + 4):
                    emit_a1_unit(ct, u // TB, u % TB)
            elif kind == "b":
                _, ct, qb = item
                pending = emit_b_block(ct, qb, pending)
            else:
                _, qb = item
                # C(qb) reads onorm[*][:, qb] — flush the pending block's
                # normalization (always (3, qb) here) before projecting.
                flush_norm(pending)
                pending = None
                for tt in range(4 * qb, 4 * qb + 4):
                    emit_c(tt)

    nc.compile()
    return nc


import ml_dtypes


def _bf16(a):
    return np.ascontiguousarray(np.asarray(a, dtype=np.float32)).astype(
        ml_dtypes.bfloat16
    )


def _make_in_maps(x, w_attn, b_attn, w_proj, include_bias):
    in_maps = []
    for i in range(N_CORES):
        b, g = divmod(i, G)
        m = {
            "xT": _bf16(x[b].T),
            "wq": _bf16(w_attn[:, 0 * C + g * CH : 0 * C + (g + 1) * CH]),
            "wk": _bf16(w_attn[:, 1 * C + g * CH : 1 * C + (g + 1) * CH]),
            "wv": _bf16(w_attn[:, 2 * C + g * CH : 2 * C + (g + 1) * CH]),
            "wo": _bf16(w_proj[g * CH : (g + 1) * CH, :]),
        }
        if include_bias:
            m["bq"] = _bf16(b_attn[0 * C + g * CH : 0 * C + (g + 1) * CH])
            m["bk"] = _bf16(b_attn[1 * C + g * CH : 1 * C + (g + 1) * CH])
            m["bv"] = _bf16(b_attn[2 * C + g * CH : 2 * C + (g + 1) * CH])
        in_maps.append(m)
    return in_maps


def kernel(**inputs) -> np.ndarray:
    global _last_results
    x = np.asarray(inputs["x"], dtype=np.float32)
    w_attn = np.asarray(inputs["w_attn"], dtype=np.float32)
    b_attn = np.asarray(inputs["b_attn"], dtype=np.float32)
    w_proj = np.asarray(inputs["w_proj"], dtype=np.float32)
    b_proj = np.asarray(inputs["b_proj"], dtype=np.float32)

    include_bias = bool(np.any(b_attn))
    nc = _build_program(include_bias)
    in_maps = _make_in_maps(x, w_attn, b_attn, w_proj, include_bias)
    res = run_bass_kernel_spmd(nc, in_maps, core_ids=list(range(N_CORES)))
    _last_results = res

    out = np.zeros((B, T, C), dtype=np.float32)
    for i in range(N_CORES):
        out[i // G] += res.results[i]["out"]
    out += b_proj
    return out


# revision 34
# speedup vs baseline: 1.1813x; 1.1813x over previous
"""Causal self-attention Trainium2 kernel.

B=4, T=2048, C=1024, H=16 heads (D=64). 8 NeuronCores.

Sharding (hybrid data/tensor parallel, Megatron-style):
  core i -> (batch b = i//2, head-group g = i%2 of 8 heads).
  c_attn column-parallel (each core owns its group's q/k/v columns),
  c_proj row-parallel (each core owns its group's rows); the 2 partial
  outputs per batch are summed on the host (host-side all-reduce),
  b_proj added once at the end.

Per-core device kernel (T=2048 tokens, 8 heads, D=64):
  A1: qT/kT produced in [D, T] layout (weights stationary, xT streaming).
  A2: V produced interleaved [tok, d|1, h] with a ones column per head
      (softmax denominators fused into PV as an extra output row).
  B:  per (head pair, q block): S^T[k,q] tiles = kT.T @ qT (K=64 matmul,
      two heads packed in row groups 0-63 / 64-127), exp on ScalarE
      straight out of PSUM (no max subtraction: logits are ~N(0,1)),
      causal masking via one gpsimd affine_select per diagonal half
      (zero-fills both the fully-masked strip and the triangle), then
      O^T_aug[d|denom, q] += [V|1].T @ P^T accumulated over k tiles.
      Normalization: 1/denom via the fast custom-DVE reciprocal
      (reciprocal_approx_fast), gpsimd partition_broadcast, DVE mul.
  C:  out[t, c] = Onorm^T.T @ wo accumulated over 4 channel tiles.

Emission order pipelines the phases so TensorE never idles long enough
for the HAM clock gate to re-throttle: A1(ct0) first, then qb-major
B blocks with the A2 chunk for that qb ahead of them, A1(ct) woven in
right before its first B block, and C chunks emitted as soon as the
last head pair of a q block completes.
"""

import sys

import numpy as np

sys.path.insert(0, "/opt/trn_rl_repo")

from contextlib import ExitStack

import concourse.bacc as bacc
import concourse.tile as tile
from concourse import mybir
from concourse.bass_utils import run_bass_kernel_spmd

F32 = mybir.dt.float32
BF16 = mybir.dt.bfloat16

B, T, C, H = 4, 2048, 1024, 16
D = C // H            # 64 head dim
G = 2                 # head groups (cores per batch)
NH = H // G           # 8 heads per core
CH = NH * D           # 512 channels per core
N_CORES = B * G       # 8

KT = C // 128         # 8 contraction tiles for qkv proj
TB = T // 512         # 4 token blocks of 512
CT = NH // 2          # 4 channel tiles (head pairs)
TT = T // 128         # 16 token tiles of 128
CB = C // 512         # 2 output channel blocks
QB = T // 512         # 4 q blocks
SCALE = 1.0 / float(np.sqrt(D))

_last_results = None  # BassKernelResults of the most recent kernel() call


def _build_program(include_bias: bool) -> bacc.Bacc:
    nc = bacc.Bacc("TRN2")

    xT = nc.dram_tensor("xT", [C, T], BF16, kind="ExternalInput").ap()
    wq = nc.dram_tensor("wq", [C, CH], BF16, kind="ExternalInput").ap()
    wk = nc.dram_tensor("wk", [C, CH], BF16, kind="ExternalInput").ap()
    wv = nc.dram_tensor("wv", [C, CH], BF16, kind="ExternalInput").ap()
    wo = nc.dram_tensor("wo", [CH, C], BF16, kind="ExternalInput").ap()
    if include_bias:
        bq = nc.dram_tensor("bq", [CH], BF16, kind="ExternalInput").ap()
        bk = nc.dram_tensor("bk", [CH], BF16, kind="ExternalInput").ap()
        bv = nc.dram_tensor("bv", [CH], BF16, kind="ExternalInput").ap()
    out = nc.dram_tensor("out", [T, C], F32, kind="ExternalOutput").ap()

    with tile.TileContext(nc) as tc, ExitStack() as ctx:
        persist = ctx.enter_context(tc.tile_pool(name="persist", bufs=1))
        # [D, T] layouts, one tile per head pair: rows 0-63 head 2*ct,
        # rows 64-127 head 2*ct+1.
        qT = [persist.tile([128, T], BF16, name=f"qT{i}", tag=f"qT{i}") for i in range(CT)]
        kTs = [persist.tile([128, T], BF16, name=f"kT{i}", tag=f"kT{i}") for i in range(CT)]
        # V interleaved h-major: vint[tt][p, h, d] = V[t=128*tt+p, head h,
        # dim d], with vint[tt][p, h, D] = 1.0 (denominator column). Head
        # slices [:, h, :] are contiguous so PV's LDWEIGHTS streams fast.
        vint = [persist.tile([128, NH, D + 1], BF16, name=f"v{i}", tag=f"v{i}") for i in range(TT)]
        # Normalized attention output, [ch, T] layout per head pair.
        onorm = [persist.tile([128, T], BF16, name=f"on{i}", tag=f"on{i}") for i in range(CT)]
        ones_row = persist.tile([1, 512], BF16, name="ones", tag="ones")
        nc.vector.memset(ones_row, 1.0)
        if include_bias:
            bias_sb = persist.tile([1, 3, CH], BF16, name="bias", tag="bias")
            nc.sync.dma_start(
                out=bias_sb[:, 0, :], in_=bq.rearrange("(a c) -> a c", a=1)
            )
            nc.sync.dma_start(
                out=bias_sb[:, 1, :], in_=bk.rearrange("(a c) -> a c", a=1)
            )
            nc.sync.dma_start(
                out=bias_sb[:, 2, :], in_=bv.rearrange("(a c) -> a c", a=1)
            )

        # Weight + xT residency (everything stays in SBUF for the whole
        # kernel so projection matmuls can interleave with attention).
        xT_sb = [
            persist.tile([128, T], BF16, name=f"xT{k}", tag=f"xT{k}")
            for k in range(KT)
        ]
        wq_sb = [persist.tile([128, CH], BF16, name=f"wq{k}", tag=f"wq{k}") for k in range(KT)]
        wk_sb = [persist.tile([128, CH], BF16, name=f"wk{k}", tag=f"wk{k}") for k in range(KT)]
        wv_sb = [persist.tile([128, CH], BF16, name=f"wv{k}", tag=f"wv{k}") for k in range(KT)]
        wo_sb = [
            persist.tile([128, C], BF16, name=f"wo{i}", tag=f"wo{i}")
            for i in range(CT)
        ]
        # DMA order: the A1(ct0) k-chunks first so its matmuls start early.
        # Input DMAs: the prologue is aggregate-bandwidth-bound (~230GB/s
        # across all queues), so splitting further doesn't help; xT rides
        # the SWDGE queue, weights the sync queue (verified fastest).
        for k in range(KT):
            nc.gpsimd.dma_start(out=xT_sb[k], in_=xT[k * 128 : (k + 1) * 128, :])
            nc.sync.dma_start(out=wq_sb[k], in_=wq[k * 128 : (k + 1) * 128, :])
            nc.sync.dma_start(out=wk_sb[k], in_=wk[k * 128 : (k + 1) * 128, :])
        for k in range(KT):
            nc.sync.dma_start(out=wv_sb[k], in_=wv[k * 128 : (k + 1) * 128, :])
        for ct in range(CT):
            nc.sync.dma_start(
                out=wo_sb[ct], in_=wo[ct * 128 : (ct + 1) * 128, :]
            )
        for tt in range(TT):
            nc.gpsimd.memset(vint[tt][:, :, D], 1.0)
        # Preload the exp activation table during the DMA prologue so the
        # first real exp doesn't pay the ~2.7us table load.
        warm_act = persist.tile([1, 8], BF16, name="wact", tag="wact")
        nc.scalar.activation(
            warm_act, ones_row[:, 0:8], mybir.ActivationFunctionType.Exp
        )

        # Shared PSUM pools. Budget (8 banks): spool 2x2 + opool 2x1 +
        # aux 2x1 = 8.
        auxps = ctx.enter_context(tc.tile_pool(name="auxps", bufs=2, space="PSUM"))
        spool = ctx.enter_context(tc.tile_pool(name="spool", bufs=2, space="PSUM"))
        opool = ctx.enter_context(tc.tile_pool(name="opool", bufs=2, space="PSUM"))
        ptpool = ctx.enter_context(tc.tile_pool(name="ptpool", bufs=16))
        rpool = ctx.enter_context(tc.tile_pool(name="rpool", bufs=5))
        bcpool = ctx.enter_context(tc.tile_pool(name="bcpool", bufs=3))
        stpool = ctx.enter_context(tc.tile_pool(name="stpool", bufs=3))
        ostage = ctx.enter_context(tc.tile_pool(name="ostage", bufs=4))
        costage = ctx.enter_context(tc.tile_pool(name="costage", bufs=3))

        def emit_a1_unit(ct, bi, tb):
            # One qT/kT 512-token block for head pair ct; weights reloaded
            # per block (LDWEIGHTS hides in the background weight buffer).
            wsb, dest = ((wq_sb, qT), (wk_sb, kTs))[bi]
            ps = auxps.tile([128, 512], F32, name="a1", tag="aux")
            for k in range(KT):
                nc.tensor.matmul(
                    ps,
                    lhsT=wsb[k][:, ct * 128 : (ct + 1) * 128],
                    rhs=xT_sb[k][:, tb * 512 : (tb + 1) * 512],
                    start=(k == 0),
                    stop=(k == KT - 1 and not include_bias),
                )
            if include_bias:
                nc.tensor.matmul(
                    ps,
                    lhsT=bias_sb[:, bi, ct * 128 : (ct + 1) * 128],
                    rhs=ones_row,
                    start=False,
                    stop=True,
                )
            nc.vector.tensor_copy(dest[ct][:, tb * 512 : (tb + 1) * 512], ps)

        def emit_a2(tt):
            # V chunk for token tile tt, interleaved layout + ones column.
            ps = auxps.tile([128, 512], F32, name="a2", tag="aux")
            for k in range(KT):
                nc.tensor.matmul(
                    ps,
                    lhsT=xT_sb[k][:, tt * 128 : (tt + 1) * 128],
                    rhs=wv_sb[k],
                    start=(k == 0),
                    stop=(k == KT - 1 and not include_bias),
                )
            if include_bias:
                nc.tensor.matmul(
                    ps,
                    lhsT=ones_row[:, 0:128],
                    rhs=bias_sb[:, 2, :],
                    start=False,
                    stop=True,
                )
            nc.vector.tensor_copy(
                vint[tt][:, :, 0:D],
                ps.rearrange("p (h d) -> p h d", h=NH),
            )

        def emit_s(ct, qb, kp, pts):
            # S^T matmuls for both heads of the pair; per k-tile the two
            # K=64 matmuls land in different PE row groups.
            ps_pair = []
            for hh in range(2):
                ps_pair.append(spool.tile([128, 1024], F32, name="s", tag="s"))
            for half in range(2):
                kt = 2 * kp + half
                j = kt - 4 * qb
                # Diagonal tiles: q columns < 128j are fully masked — skip
                # them in the matmul (the affine_select already treats that
                # region as a fill zone, so downstream logic is unchanged).
                off = 128 * j if j > 0 else 0
                for hh in range(2):
                    rb = 64 * hh
                    nc.tensor.matmul(
                        ps_pair[hh][:, half * 512 + off : (half + 1) * 512],
                        lhsT=kTs[ct][rb : rb + 64, kt * 128 : (kt + 1) * 128],
                        rhs=qT[ct][rb : rb + 64, qb * 512 + off : (qb + 1) * 512],
                        start=True,
                        stop=True,
                        tile_position=(rb, 0),
                    )
            j0 = 2 * kp - 4 * qb  # diag offset of first half (<0: below)
            for hh in range(2):
                ps_s = ps_pair[hh]
                pt = ptpool.tile([128, 1024], BF16, name="pt", tag="pt")
                if j0 <= 0:
                    # Fully below the diagonal (j0 < 0), or diag pair A
                    # (j0 == 0: only 128 masked cols — cheaper to exp them
                    # and zero-fill than to split the ACT).
                    nc.scalar.activation(
                        pt, ps_s, mybir.ActivationFunctionType.Exp,
                        scale=SCALE,
                    )
                else:
                    # diag pair B (j0 == 2): halves j=2, j=3; skip the
                    # large fully-masked strips in the ACT.
                    nc.scalar.activation(
                        pt[:, 256:512], ps_s[:, 256:512],
                        mybir.ActivationFunctionType.Exp, scale=SCALE,
                    )
                    nc.scalar.activation(
                        pt[:, 896:1024], ps_s[:, 896:1024],
                        mybir.ActivationFunctionType.Exp, scale=SCALE,
                    )
                if j0 >= 0:
                    # Triangle-only select: PV's N-trim skips the fully
                    # masked strip (cols < 128j of the half), so only the
                    # 128-wide diagonal chunk needs masking — keep
                    # pt[ch, c] iff c - ch >= 0 within the chunk.
                    for half in range(2):
                        j = j0 + half
                        o = half * 512 + 128 * j
                        nc.gpsimd.affine_select(
                            out=pt[:, o : o + 128],
                            in_=pt[:, o : o + 128],
                            compare_op=mybir.AluOpType.is_ge,
                            fill=0.0,
                            base=0,
                            channel_multiplier=-1,
                            pattern=[[1, 128]],
                        )
                pts[(kp, hh)] = pt

        def emit_pv(ct, qb, kp, nkt, oaug, pts):
            for hh in range(2):
                h = 2 * ct + hh
                pt = pts.pop((kp, hh))
                for half in range(2):
                    kt = 2 * kp + half
                    j = kt - 4 * qb
                    # Diagonal tiles contribute nothing to q cols < 128j;
                    # skip those output columns (they were initialized by
                    # the kt==0 matmul, which is never trimmed).
                    off = 128 * j if j > 0 else 0
                    nc.tensor.matmul(
                        oaug[hh][:, off:512],
                        lhsT=vint[kt][:, h, :],
                        rhs=pt[:, half * 512 + off : (half + 1) * 512],
                        start=(kt == 0),
                        stop=(kt == nkt - 1),
                    )

        def flush_norm(pending):
            # Normalization of an earlier block, deferred so its gpsimd
            # partition_broadcast queues BEHIND the next block's
            # affine_selects (no head-of-line blocking of PV).
            if pending is None:
                return
            ocps, ct, qb = pending
            qs = slice(qb * 512, (qb + 1) * 512)
            for hh in range(2):
                ocp, dstg = ocps[hh]
                rc = rpool.tile([1, 512], F32, name="r", tag="r")
                nc.vector.reciprocal_approx_fast(rc, dstg)
                bc = bcpool.tile([64, 512], F32, name="bc", tag="bc")
                nc.gpsimd.partition_broadcast(bc, rc, channels=64)
                if hh == 0:
                    nc.vector.tensor_mul(
                        onorm[ct][0:64, qs], ocp[0:D, :], bc
                    )
                else:
                    stg = stpool.tile([64, 512], BF16, name="st", tag="st")
                    nc.vector.tensor_mul(stg, ocp[0:D, :], bc)
                    nc.sync.dma_start(out=onorm[ct][64:128, qs], in_=stg)

        def emit_b_block(ct, qb, pending):
            nkt = 4 * qb + 4  # causal: only k tiles with k <= q
            nkp = nkt // 2
            # Strict mode separation: all 64-row S matmuls first (T0/T8
            # row-tile concurrency), then all 128-row PV matmuls —
            # interleaving the two tile modes forces a TensorE drain per
            # switch, and the Tile scheduler re-interleaves by readiness
            # anyway (verified fastest of the orderings tried).
            pts = {}
            for kp in range(nkp):
                emit_s(ct, qb, kp, pts)
            flush_norm(pending)
            if ct == 0:
                # V chunk for this q block, needed by PV(0, qb) onward.
                for tt in range(4 * qb, 4 * qb + 4):
                    emit_a2(tt)
            oaug = [
                opool.tile([D + 1, 512], F32, name=f"oaug{hh}", tag="oaug")
                for hh in range(2)
            ]
            for kp in range(nkp):
                emit_pv(ct, qb, kp, nkt, oaug, pts)
            # Evacuate O_aug to SBUF immediately: frees the PSUM bank for
            # the next block; normalization runs off the SBUF copy later.
            # Evacuate to SBUF immediately (frees the PSUM bank for the
            # next block): O rows partition-aligned, the denominator row
            # via a separate plain copy (partition 64 -> 0; the custom-DVE
            # reciprocal cannot handle a cross-partition source itself).
            ocps = []
            for hh in range(2):
                ocp = ostage.tile([D, 512], F32, name="oc", tag="oc")
                nc.vector.tensor_copy(ocp, oaug[hh][0:D, :])
                dstg = rpool.tile([1, 512], F32, name="d", tag="d")
                nc.vector.tensor_copy(dstg, oaug[hh][D : D + 1, :])
                ocps.append((ocp, dstg))
            return (ocps, ct, qb)

        def emit_c(tt):
            # out[tt block] = sum_ct onorm[ct]^T @ wo[ct]
            pcs = [
                auxps.tile([128, 512], F32, name="c", tag="aux")
                for _ in range(CB)
            ]
            for ct in range(CT):
                for cb in range(CB):
                    nc.tensor.matmul(
                        pcs[cb],
                        lhsT=onorm[ct][:, tt * 128 : (tt + 1) * 128],
                        rhs=wo_sb[ct][:, cb * 512 : (cb + 1) * 512],
                        start=(ct == 0),
                        stop=(ct == CT - 1),
                    )
            for cb in range(CB):
                ot = costage.tile([128, 512], F32, name="o", tag="o")
                # Split the PSUM evacuations between the two engines that
                # can read PSUM so neither becomes the pole.
                if cb == 0:
                    nc.vector.tensor_copy(ot, pcs[cb])
                else:
                    nc.scalar.copy(ot, pcs[cb])
                nc.sync.dma_start(
                    out=out[
                        tt * 128 : (tt + 1) * 128,
                        cb * 512 : (cb + 1) * 512,
                    ],
                    in_=ot,
                )

        # ---------------- pipelined emission ----------------
        # Zig-zag block order, tuned so the cumulative exp supply to
        # ScalarE tracks cumulative TensorE work: A1(0) units pairwise
        # feed B(0,qb) blocks (first exp ~DMA-bound), A1(ct) quads woven
        # as filler, late B blocks ordered large-qb-first to keep the exp
        # stream dense, C chunks as soon as their last block lands.
        SEQ = [
            ("u", 0, 0, 0), ("u", 0, 1, 0), ("b", 0, 0),
            ("u", 0, 0, 1), ("u", 0, 1, 1), ("b", 0, 1),
            ("uq", 1, 0),
            ("u", 0, 0, 2), ("u", 0, 1, 2), ("b", 0, 2),
            ("uq", 1, 1),
            ("u", 0, 0, 3), ("u", 0, 1, 3), ("b", 0, 3),
            ("b", 1, 3), ("b", 1, 2),
            ("uq", 2, 0),
            ("b", 1, 1),
            ("uq", 2, 1),
            ("b", 1, 0),
            ("b", 2, 3),
            ("uq", 3, 0),
            ("b", 2, 2),
            ("uq", 3, 1),
            ("b", 3, 3), ("c", 3),
            ("b", 2, 1),
            ("b", 3, 2), ("c", 2),
            ("b", 2, 0),
            ("b", 3, 1), ("c", 1),
            ("b", 3, 0), ("c", 0),
        ]
        pending = None
        for item in SEQ:
            kind = item[0]
            if kind == "u":
                _, ct, bi, tb = item
                emit_a1_unit(ct, bi, tb)
            elif kind == "uq":
                _, ct, q = item
                for u in range(4 * q, 4 * q + 4):
                    emit_a1_unit(ct, u // TB, u % TB)
            elif kind == "b":
                _, ct, qb = item
                pending = emit_b_block(ct, qb, pending)
            else:
                _, qb = item
                # C(qb) reads onorm[*][:, qb] — flush the pending block's
                # normalization (always (3, qb) here) before projecting.
                flush_norm(pending)
                pending = None
                for tt in range(4 * qb, 4 * qb + 4):
                    emit_c(tt)

    nc.compile()
    return nc


import ml_dtypes


def _bf16(a):
    return np.ascontiguousarray(np.asarray(a, dtype=np.float32)).astype(
        ml_dtypes.bfloat16
    )


def _make_in_maps(x, w_attn, b_attn, w_proj, include_bias):
    in_maps = []
    for i in range(N_CORES):
        b, g = divmod(i, G)
        m = {
            "xT": _bf16(x[b].T),
            "wq": _bf16(w_attn[:, 0 * C + g * CH : 0 * C + (g + 1) * CH]),
            "wk": _bf16(w_attn[:, 1 * C + g * CH : 1 * C + (g + 1) * CH]),
            "wv": _bf16(w_attn[:, 2 * C + g * CH : 2 * C + (g + 1) * CH]),
            "wo": _bf16(w_proj[g * CH : (g + 1) * CH, :]),
        }
        if include_bias:
            m["bq"] = _bf16(b_attn[0 * C + g * CH : 0 * C + (g + 1) * CH])
            m["bk"] = _bf16(b_attn[1 * C + g * CH : 1 * C + (g + 1) * CH])
            m["bv"] = _bf16(b_attn[2 * C + g * CH : 2 * C + (g + 1) * CH])
        in_maps.append(m)
    return in_maps


def kernel(**inputs) -> np.ndarray:
    global _last_results
    x = np.asarray(inputs["x"], dtype=np.float32)
    w_attn = np.asarray(inputs["w_attn"], dtype=np.float32)
    b_attn = np.asarray(inputs["b_attn"], dtype=np.float32)
    w_proj = np.asarray(inputs["w_proj"], dtype=np.float32)
    b_proj = np.asarray(inputs["b_proj"], dtype=np.float32)

    include_bias = bool(np.any(b_attn))
    nc = _build_program(include_bias)
    in_maps = _make_in_maps(x, w_attn, b_attn, w_proj, include_bias)
    res = run_bass_kernel_spmd(nc, in_maps, core_ids=list(range(N_CORES)))
    _last_results = res

    out = np.zeros((B, T, C), dtype=np.float32)
    for i in range(N_CORES):
        out[i // G] += res.results[i]["out"]
    out += b_proj
    return out


# revision 35
# speedup vs baseline: 1.2214x; 1.0339x over previous
"""Causal self-attention Trainium2 kernel.

B=4, T=2048, C=1024, H=16 heads (D=64). 8 NeuronCores.

Sharding (hybrid data/tensor parallel, Megatron-style):
  core i -> (batch b = i//2, head-group g = i%2 of 8 heads).
  c_attn column-parallel (each core owns its group's q/k/v columns),
  c_proj row-parallel (each core owns its group's rows); the 2 partial
  outputs per batch are summed on the host (host-side all-reduce),
  b_proj added once at the end.

Per-core device kernel (T=2048 tokens, 8 heads, D=64):
  A1: qT/kT produced in [D, T] layout (weights stationary, xT streaming).
  A2: V produced interleaved [tok, d|1, h] with a ones column per head
      (softmax denominators fused into PV as an extra output row).
  B:  per (head pair, q block): S^T[k,q] tiles = kT.T @ qT (K=64 matmul,
      two heads packed in row groups 0-63 / 64-127), exp on ScalarE
      straight out of PSUM (no max subtraction: logits are ~N(0,1)),
      causal masking via one gpsimd affine_select per diagonal half
      (zero-fills both the fully-masked strip and the triangle), then
      O^T_aug[d|denom, q] += [V|1].T @ P^T accumulated over k tiles.
      Normalization: 1/denom via the fast custom-DVE reciprocal
      (reciprocal_approx_fast), gpsimd partition_broadcast, DVE mul.
  C:  out[t, c] = Onorm^T.T @ wo accumulated over 4 channel tiles.

Emission order pipelines the phases so TensorE never idles long enough
for the HAM clock gate to re-throttle: A1(ct0) first, then qb-major
B blocks with the A2 chunk for that qb ahead of them, A1(ct) woven in
right before its first B block, and C chunks emitted as soon as the
last head pair of a q block completes.
"""

import sys

import numpy as np

sys.path.insert(0, "/opt/trn_rl_repo")

from contextlib import ExitStack

import concourse.bacc as bacc
import concourse.tile as tile
from concourse import mybir
from concourse.bass_utils import run_bass_kernel_spmd

F32 = mybir.dt.float32
BF16 = mybir.dt.bfloat16

B, T, C, H = 4, 2048, 1024, 16
D = C // H            # 64 head dim
G = 2                 # head groups (cores per batch)
NH = H // G           # 8 heads per core
CH = NH * D           # 512 channels per core
N_CORES = B * G       # 8

KT = C // 128         # 8 contraction tiles for qkv proj
TB = T // 512         # 4 token blocks of 512
CT = NH // 2          # 4 channel tiles (head pairs)
TT = T // 128         # 16 token tiles of 128
CB = C // 512         # 2 output channel blocks
QB = T // 512         # 4 q blocks
SCALE = 1.0 / float(np.sqrt(D))

_last_results = None  # BassKernelResults of the most recent kernel() call


def _build_program(include_bias: bool) -> bacc.Bacc:
    nc = bacc.Bacc("TRN2")

    xT = nc.dram_tensor("xT", [C, T], BF16, kind="ExternalInput").ap()
    wq = nc.dram_tensor("wq", [C, CH], BF16, kind="ExternalInput").ap()
    wk = nc.dram_tensor("wk", [C, CH], BF16, kind="ExternalInput").ap()
    wv = nc.dram_tensor("wv", [C, CH], BF16, kind="ExternalInput").ap()
    wo = nc.dram_tensor("wo", [CH, C], BF16, kind="ExternalInput").ap()
    if include_bias:
        bq = nc.dram_tensor("bq", [CH], BF16, kind="ExternalInput").ap()
        bk = nc.dram_tensor("bk", [CH], BF16, kind="ExternalInput").ap()
        bv = nc.dram_tensor("bv", [CH], BF16, kind="ExternalInput").ap()
    out = nc.dram_tensor("out", [T, C], F32, kind="ExternalOutput").ap()

    with tile.TileContext(nc) as tc, ExitStack() as ctx:
        persist = ctx.enter_context(tc.tile_pool(name="persist", bufs=1))
        # [D, T] layouts, one tile per head pair: rows 0-63 head 2*ct,
        # rows 64-127 head 2*ct+1.
        qT = [persist.tile([128, T], BF16, name=f"qT{i}", tag=f"qT{i}") for i in range(CT)]
        kTs = [persist.tile([128, T], BF16, name=f"kT{i}", tag=f"kT{i}") for i in range(CT)]
        # V interleaved h-major: vint[tt][p, h, d] = V[t=128*tt+p, head h,
        # dim d], with vint[tt][p, h, D] = 1.0 (denominator column). Head
        # slices [:, h, :] are contiguous so PV's LDWEIGHTS streams fast.
        vint = [persist.tile([128, NH, D + 1], BF16, name=f"v{i}", tag=f"v{i}") for i in range(TT)]
        # Normalized attention output, [ch, T] layout per head pair.
        onorm = [persist.tile([128, T], BF16, name=f"on{i}", tag=f"on{i}") for i in range(CT)]
        ones_row = persist.tile([1, 512], BF16, name="ones", tag="ones")
        nc.vector.memset(ones_row, 1.0)
        if include_bias:
            bias_sb = persist.tile([1, 3, CH], BF16, name="bias", tag="bias")
            nc.sync.dma_start(
                out=bias_sb[:, 0, :], in_=bq.rearrange("(a c) -> a c", a=1)
            )
            nc.sync.dma_start(
                out=bias_sb[:, 1, :], in_=bk.rearrange("(a c) -> a c", a=1)
            )
            nc.sync.dma_start(
                out=bias_sb[:, 2, :], in_=bv.rearrange("(a c) -> a c", a=1)
            )

        # Weight + xT residency (everything stays in SBUF for the whole
        # kernel so projection matmuls can interleave with attention).
        xT_sb = [
            persist.tile([128, T], BF16, name=f"xT{k}", tag=f"xT{k}")
            for k in range(KT)
        ]
        wq_sb = [persist.tile([128, CH], BF16, name=f"wq{k}", tag=f"wq{k}") for k in range(KT)]
        wk_sb = [persist.tile([128, CH], BF16, name=f"wk{k}", tag=f"wk{k}") for k in range(KT)]
        wv_sb = [persist.tile([128, CH], BF16, name=f"wv{k}", tag=f"wv{k}") for k in range(KT)]
        wo_sb = [
            persist.tile([128, C], BF16, name=f"wo{i}", tag=f"wo{i}")
            for i in range(CT)
        ]
        # DMA order: the A1(ct0) k-chunks first so its matmuls start early.
        # Input DMAs: the prologue is aggregate-bandwidth-bound (~230GB/s
        # across all queues), so splitting further doesn't help; xT rides
        # the SWDGE queue, weights the sync queue (verified fastest).
        for k in range(KT):
            nc.gpsimd.dma_start(out=xT_sb[k], in_=xT[k * 128 : (k + 1) * 128, :])
            nc.sync.dma_start(out=wq_sb[k], in_=wq[k * 128 : (k + 1) * 128, :])
            nc.sync.dma_start(out=wk_sb[k], in_=wk[k * 128 : (k + 1) * 128, :])
        for k in range(KT):
            nc.sync.dma_start(out=wv_sb[k], in_=wv[k * 128 : (k + 1) * 128, :])
        for ct in range(CT):
            nc.sync.dma_start(
                out=wo_sb[ct], in_=wo[ct * 128 : (ct + 1) * 128, :]
            )
        for tt in range(TT):
            nc.gpsimd.memset(vint[tt][:, :, D], 1.0)
        # Preload the exp activation table during the DMA prologue so the
        # first real exp doesn't pay the ~2.7us table load.
        warm_act = persist.tile([1, 8], BF16, name="wact", tag="wact")
        nc.scalar.activation(
            warm_act, ones_row[:, 0:8], mybir.ActivationFunctionType.Exp
        )

        # Shared PSUM pools. Budget (8 banks): spool 2x2 + opool 2x1 +
        # aux 2x1 = 8.
        auxps = ctx.enter_context(tc.tile_pool(name="auxps", bufs=2, space="PSUM"))
        spool = ctx.enter_context(tc.tile_pool(name="spool", bufs=2, space="PSUM"))
        opool = ctx.enter_context(tc.tile_pool(name="opool", bufs=2, space="PSUM"))
        ptpool = ctx.enter_context(tc.tile_pool(name="ptpool", bufs=16))
        rpool = ctx.enter_context(tc.tile_pool(name="rpool", bufs=5))
        bcpool = ctx.enter_context(tc.tile_pool(name="bcpool", bufs=3))
        stpool = ctx.enter_context(tc.tile_pool(name="stpool", bufs=3))
        ostage = ctx.enter_context(tc.tile_pool(name="ostage", bufs=4))
        costage = ctx.enter_context(tc.tile_pool(name="costage", bufs=3))

        def emit_a1_unit(ct, bi, tb):
            # One qT/kT 512-token block for head pair ct; weights reloaded
            # per block (LDWEIGHTS hides in the background weight buffer).
            wsb, dest = ((wq_sb, qT), (wk_sb, kTs))[bi]
            ps = auxps.tile([128, 512], F32, name="a1", tag="aux")
            for k in range(KT):
                nc.tensor.matmul(
                    ps,
                    lhsT=wsb[k][:, ct * 128 : (ct + 1) * 128],
                    rhs=xT_sb[k][:, tb * 512 : (tb + 1) * 512],
                    start=(k == 0),
                    stop=(k == KT - 1 and not include_bias),
                )
            if include_bias:
                nc.tensor.matmul(
                    ps,
                    lhsT=bias_sb[:, bi, ct * 128 : (ct + 1) * 128],
                    rhs=ones_row,
                    start=False,
                    stop=True,
                )
            nc.vector.tensor_copy(dest[ct][:, tb * 512 : (tb + 1) * 512], ps)

        def emit_a2(tt):
            # V chunk for token tile tt, interleaved layout + ones column.
            ps = auxps.tile([128, 512], F32, name="a2", tag="aux")
            for k in range(KT):
                nc.tensor.matmul(
                    ps,
                    lhsT=xT_sb[k][:, tt * 128 : (tt + 1) * 128],
                    rhs=wv_sb[k],
                    start=(k == 0),
                    stop=(k == KT - 1 and not include_bias),
                )
            if include_bias:
                nc.tensor.matmul(
                    ps,
                    lhsT=ones_row[:, 0:128],
                    rhs=bias_sb[:, 2, :],
                    start=False,
                    stop=True,
                )
            nc.vector.tensor_copy(
                vint[tt][:, :, 0:D],
                ps.rearrange("p (h d) -> p h d", h=NH),
            )

        def emit_s(ct, qb, kp, pts):
            # S^T matmuls for both heads of the pair; per k-tile the two
            # K=64 matmuls land in different PE row groups.
            ps_pair = []
            for hh in range(2):
                ps_pair.append(spool.tile([128, 1024], F32, name="s", tag="s"))
            for half in range(2):
                kt = 2 * kp + half
                j = kt - 4 * qb
                # Diagonal tiles: q columns < 128j are fully masked — skip
                # them in the matmul (the affine_select already treats that
                # region as a fill zone, so downstream logic is unchanged).
                off = 128 * j if j > 0 else 0
                for hh in range(2):
                    rb = 64 * hh
                    nc.tensor.matmul(
                        ps_pair[hh][:, half * 512 + off : (half + 1) * 512],
                        lhsT=kTs[ct][rb : rb + 64, kt * 128 : (kt + 1) * 128],
                        rhs=qT[ct][rb : rb + 64, qb * 512 + off : (qb + 1) * 512],
                        start=True,
                        stop=True,
                        tile_position=(rb, 0),
                    )
            j0 = 2 * kp - 4 * qb  # diag offset of first half (<0: below)
            for hh in range(2):
                ps_s = ps_pair[hh]
                pt = ptpool.tile([128, 1024], BF16, name="pt", tag="pt")
                if j0 <= 0:
                    # Fully below the diagonal (j0 < 0), or diag pair A
                    # (j0 == 0: only 128 masked cols — cheaper to exp them
                    # and zero-fill than to split the ACT).
                    nc.scalar.activation(
                        pt, ps_s, mybir.ActivationFunctionType.Exp,
                        scale=SCALE,
                    )
                else:
                    # diag pair B (j0 == 2): halves j=2, j=3; skip the
                    # large fully-masked strips in the ACT.
                    nc.scalar.activation(
                        pt[:, 256:512], ps_s[:, 256:512],
                        mybir.ActivationFunctionType.Exp, scale=SCALE,
                    )
                    nc.scalar.activation(
                        pt[:, 896:1024], ps_s[:, 896:1024],
                        mybir.ActivationFunctionType.Exp, scale=SCALE,
                    )
                if j0 >= 0:
                    # Triangle-only select: PV's N-trim skips the fully
                    # masked strip (cols < 128j of the half), so only the
                    # 128-wide diagonal chunk needs masking — keep
                    # pt[ch, c] iff c - ch >= 0 within the chunk.
                    for half in range(2):
                        j = j0 + half
                        o = half * 512 + 128 * j
                        nc.gpsimd.affine_select(
                            out=pt[:, o : o + 128],
                            in_=pt[:, o : o + 128],
                            compare_op=mybir.AluOpType.is_ge,
                            fill=0.0,
                            base=0,
                            channel_multiplier=-1,
                            pattern=[[1, 128]],
                        )
                pts[(kp, hh)] = pt

        def emit_pv(ct, qb, kp, nkt, oaug, pts):
            for hh in range(2):
                h = 2 * ct + hh
                pt = pts.pop((kp, hh))
                for half in range(2):
                    kt = 2 * kp + half
                    j = kt - 4 * qb
                    # Diagonal tiles contribute nothing to q cols < 128j;
                    # skip those output columns (they were initialized by
                    # the kt==0 matmul, which is never trimmed).
                    off = 128 * j if j > 0 else 0
                    nc.tensor.matmul(
                        oaug[hh][:, off:512],
                        lhsT=vint[kt][:, h, :],
                        rhs=pt[:, half * 512 + off : (half + 1) * 512],
                        start=(kt == 0),
                        stop=(kt == nkt - 1),
                    )

        def flush_norm(pending):
            # Normalization of an earlier block, deferred so its gpsimd
            # partition_broadcast queues BEHIND the next block's
            # affine_selects (no head-of-line blocking of PV).
            if pending is None:
                return
            ocps, ct, qb = pending
            qs = slice(qb * 512, (qb + 1) * 512)
            for hh in range(2):
                ocp, dstg = ocps[hh]
                rc = rpool.tile([1, 512], F32, name="r", tag="r")
                nc.vector.reciprocal_approx_fast(rc, dstg)
                bc = bcpool.tile([64, 512], F32, name="bc", tag="bc")
                nc.gpsimd.partition_broadcast(bc, rc, channels=64)
                if hh == 0:
                    nc.vector.tensor_mul(
                        onorm[ct][0:64, qs], ocp[0:D, :], bc
                    )
                else:
                    stg = stpool.tile([64, 512], BF16, name="st", tag="st")
                    nc.vector.tensor_mul(stg, ocp[0:D, :], bc)
                    nc.sync.dma_start(out=onorm[ct][64:128, qs], in_=stg)

        def emit_b_block(ct, qb, pending):
            nkt = 4 * qb + 4  # causal: only k tiles with k <= q
            nkp = nkt // 2
            # Strict mode separation: all 64-row S matmuls first (T0/T8
            # row-tile concurrency), then all 128-row PV matmuls —
            # interleaving the two tile modes forces a TensorE drain per
            # switch, and the Tile scheduler re-interleaves by readiness
            # anyway (verified fastest of the orderings tried).
            pts = {}
            for kp in range(nkp):
                emit_s(ct, qb, kp, pts)
            flush_norm(pending)
            if ct == 0:
                # V chunk for this q block, needed by PV(0, qb) onward.
                for tt in range(4 * qb, 4 * qb + 4):
                    emit_a2(tt)
            oaug = [
                opool.tile([D + 1, 512], F32, name=f"oaug{hh}", tag="oaug")
                for hh in range(2)
            ]
            for kp in range(nkp):
                emit_pv(ct, qb, kp, nkt, oaug, pts)
            # Evacuate O_aug to SBUF immediately: frees the PSUM bank for
            # the next block; normalization runs off the SBUF copy later.
            # Evacuate to SBUF immediately (frees the PSUM bank for the
            # next block): O rows partition-aligned, the denominator row
            # via a separate plain copy (partition 64 -> 0; the custom-DVE
            # reciprocal cannot handle a cross-partition source itself).
            ocps = []
            for hh in range(2):
                ocp = ostage.tile([D, 512], F32, name="oc", tag="oc")
                nc.vector.tensor_copy(ocp, oaug[hh][0:D, :])
                dstg = rpool.tile([1, 512], F32, name="d", tag="d")
                nc.vector.tensor_copy(dstg, oaug[hh][D : D + 1, :])
                ocps.append((ocp, dstg))
            return (ocps, ct, qb)

        def emit_c(tt):
            # out[tt block] = sum_ct onorm[ct]^T @ wo[ct]. cb-outer so
            # each PSUM tile's evacuation hides under the next cb's
            # matmul chain: with the 2-slot aux ring, allocating both cb
            # tiles up front made the next tt's first matmul wait ~1.2us
            # for the copies.
            for cb in range(CB):
                pc = auxps.tile([128, 512], F32, name="c", tag="aux")
                for ct in range(CT):
                    nc.tensor.matmul(
                        pc,
                        lhsT=onorm[ct][:, tt * 128 : (tt + 1) * 128],
                        rhs=wo_sb[ct][:, cb * 512 : (cb + 1) * 512],
                        start=(ct == 0),
                        stop=(ct == CT - 1),
                    )
                ot = costage.tile([128, 512], F32, name="o", tag="o")
                # Split the PSUM evacuations between the two engines that
                # can read PSUM so neither becomes the pole.
                if cb == 0:
                    nc.vector.tensor_copy(ot, pc)
                else:
                    nc.scalar.copy(ot, pc)
                nc.sync.dma_start(
                    out=out[
                        tt * 128 : (tt + 1) * 128,
                        cb * 512 : (cb + 1) * 512,
                    ],
                    in_=ot,
                )

        # ---------------- pipelined emission ----------------
        # Zig-zag block order, tuned so the cumulative exp supply to
        # ScalarE tracks cumulative TensorE work: A1(0) units pairwise
        # feed B(0,qb) blocks (first exp ~DMA-bound), A1(ct) quads woven
        # as filler, late B blocks ordered large-qb-first to keep the exp
        # stream dense, C chunks as soon as their last block lands.
        SEQ = [
            ("u", 0, 0, 0), ("u", 0, 1, 0), ("b", 0, 0),
            ("u", 0, 0, 1), ("u", 0, 1, 1), ("b", 0, 1),
            ("uq", 1, 0),
            ("u", 0, 0, 2), ("u", 0, 1, 2), ("b", 0, 2),
            ("uq", 1, 1),
            ("u", 0, 0, 3), ("u", 0, 1, 3), ("b", 0, 3),
            ("b", 1, 3), ("b", 1, 2),
            ("uq", 2, 0),
            ("b", 1, 1),
            ("uq", 2, 1),
            ("b", 1, 0),
            ("b", 2, 3),
            ("uq", 3, 0),
            ("b", 2, 2),
            ("uq", 3, 1),
            ("b", 3, 3), ("c", 3),
            ("b", 2, 1),
            ("b", 3, 2), ("c", 2),
            ("b", 2, 0),
            ("b", 3, 1), ("c", 1),
            ("b", 3, 0), ("c", 0),
        ]
        pending = None
        for item in SEQ:
            kind = item[0]
            if kind == "u":
                _, ct, bi, tb = item
                emit_a1_unit(ct, bi, tb)
            elif kind == "uq":
                _, ct, q = item
                for u in range(4 * q, 4 * q + 4):
                    emit_a1_unit(ct, u // TB, u % TB)
            elif kind == "b":
                _, ct, qb = item
                pending = emit_b_block(ct, qb, pending)
            else:
                _, qb = item
                # C(qb) reads onorm[*][:, qb] — flush the pending block's
                # normalization (always (3, qb) here) before projecting.
                flush_norm(pending)
                pending = None
                for tt in range(4 * qb, 4 * qb + 4):
                    emit_c(tt)

    nc.compile()
    return nc


import ml_dtypes


def _bf16(a):
    return np.ascontiguousarray(np.asarray(a, dtype=np.float32)).astype(
        ml_dtypes.bfloat16
    )


def _make_in_maps(x, w_attn, b_attn, w_proj, include_bias):
    in_maps = []
    for i in range(N_CORES):
        b, g = divmod(i, G)
        m = {
            "xT": _bf16(x[b].T),
            "wq": _bf16(w_attn[:, 0 * C + g * CH : 0 * C + (g + 1) * CH]),
            "wk": _bf16(w_attn[:, 1 * C + g * CH : 1 * C + (g + 1) * CH]),
            "wv": _bf16(w_attn[:, 2 * C + g * CH : 2 * C + (g + 1) * CH]),
            "wo": _bf16(w_proj[g * CH : (g + 1) * CH, :]),
        }
        if include_bias:
            m["bq"] = _bf16(b_attn[0 * C + g * CH : 0 * C + (g + 1) * CH])
            m["bk"] = _bf16(b_attn[1 * C + g * CH : 1 * C + (g + 1) * CH])
            m["bv"] = _bf16(b_attn[2 * C + g * CH : 2 * C + (g + 1) * CH])
        in_maps.append(m)
    return in_maps


def kernel(**inputs) -> np.ndarray:
    global _last_results
    x = np.asarray(inputs["x"], dtype=np.float32)
    w_attn = np.asarray(inputs["w_attn"], dtype=np.float32)
    b_attn = np.asarray(inputs["b_attn"], dtype=np.float32)
    w_proj = np.asarray(inputs["w_proj"], dtype=np.float32)
    b_proj = np.asarray(inputs["b_proj"], dtype=np.float32)

    include_bias = bool(np.any(b_attn))
    nc = _build_program(include_bias)
    in_maps = _make_in_maps(x, w_attn, b_attn, w_proj, include_bias)
    res = run_bass_kernel_spmd(nc, in_maps, core_ids=list(range(N_CORES)))
    _last_results = res

    out = np.zeros((B, T, C), dtype=np.float32)
    for i in range(N_CORES):
        out[i // G] += res.results[i]["out"]
    out += b_proj
    return out
